# revision 32
# baseline (speedup 1.0000x reference)
"""Two-layer GAT encoder on 8 TRN2 NeuronCores (Bass/Tile).

Strategy (graph-parallel, per sharding hint):
- Nodes are assigned to (core, block, slot) by a host-side balanced
  partition (greedy LPT on input in-degree): every block has exactly 128
  node slots and <= (T-1)*128 regular edges, so all blocks run the same
  T-tile schedule.  The host permutes x on the way in and un-permutes the
  output, so the kernel only ever sees the balanced layout.
- Per layer each core projects its node shard ([xh | al_src | al_dst] in
  one matmul pair), writes fp16 rows to a bounce buffer, and AllGathers
  them into a full table in shared DRAM.  The AllGather is split into 6
  chunks of 5 blocks, each issued as soon as its producer blocks finish,
  so the collective wire time overlaps phase-A compute (layer 1) and the
  edge-phase-1 + emit compute (layer 2).
- Edge aggregation per dst block: one dma_gather pulls 1280B rows by
  source id (self-loops come from the local bounce, tile 0).  The
  edge-major one-hot S is built on DVE (is_equal in 2x_1p mode via a
  duplicated-dst-value access pattern); the dst-major one-hot St is
  prebuilt on the host and DMAed.  Logits: one ident-matmul (al_src) plus
  per-tile St-matmuls accumulate e in PSUM; exp(leakyrelu) goes through
  ACT into a duplicated-pair wexp2 so the feature weighting multiply also
  hits DVE 2x_1p.  Scatter-add per tile is a single 512-col matmul plus a
  4-col z matmul sharing the same stationary S.
"""

import heapq

import numpy as np

import concourse.bacc as bacc
import concourse.mybir as mybir
import concourse.tile as tile
from concourse import bass_utils

N = 30000
IN = 256
HID = 128
H = 4
LAT = 128
NEG = 0.2
NCORES = 8
NBLK = 30                  # dst blocks of 128 per core
NSLOT = NBLK * 128         # 3840 node slots per core (30000 real + pad)
NT = NCORES * NSLOT        # 30720 table rows
D1 = H * HID               # 512
EXTC = D1 + 2 * H          # 520 = xh | al_src | al_dst
ROW = 640                  # table row elements (fp16) -> 1280B, 256B-aligned
CHB = 5                    # blocks per AllGather chunk
NCH = NBLK // CHB          # 6 chunks
CHROWS = CHB * 128         # 640 rows per core per chunk
F32 = mybir.dt.float32
F16 = mybir.dt.float16
I16 = mybir.dt.int16

_CACHE = {}


def _fold(W, a):
    K = W.shape[0]
    Hh, C = a.shape
    return np.einsum("khc,hc->kh", W.reshape(K, Hh, C).astype(np.float64),
                     a.astype(np.float64)).astype(np.float32)


def _build(T, add_b1, add_b2):
    nc = bacc.Bacc("TRN2", target_bir_lowering=False, debug=False,
                   num_devices=NCORES, num_swdge_queues=4)
    NI = (T - 1) * 128         # gathered (regular) edge slots per block
    NIW = NI // 16

    xT = nc.dram_tensor("xT", [IN, NSLOT], F16, kind="ExternalInput")
    w1e = nc.dram_tensor("w1e", [IN, EXTC], F16, kind="ExternalInput")
    w2e = nc.dram_tensor("w2e", [D1, EXTC], F16, kind="ExternalInput")
    srcidx = nc.dram_tensor("srcidx", [NBLK, 128, NIW], I16, kind="ExternalInput")
    dst2 = nc.dram_tensor("dst2", [NBLK, 128, T * 2], F16, kind="ExternalInput")
    stall_in = nc.dram_tensor("stall_in", [NBLK, 128, (T - 1) * 128],
                              mybir.dt.float8e4, kind="ExternalInput")
    iota_in = nc.dram_tensor("iota_in", [128, T * 128], F16, kind="ExternalInput")
    ident_in = nc.dram_tensor("ident_in", [128, 128], F16, kind="ExternalInput")
    if add_b1:
        b1rep = nc.dram_tensor("b1rep", [128, D1], F32, kind="ExternalInput")
    if add_b2:
        b2rep = nc.dram_tensor("b2rep", [128, LAT], F32, kind="ExternalInput")
    out_ext = nc.dram_tensor("out", [NSLOT, LAT], F32, kind="ExternalOutput")

    with tile.TileContext(nc) as tc:
        with (
            tc.tile_pool(name="const", bufs=1) as const,
            tc.tile_pool(name="gs", bufs=4) as gtp_g,
            tc.tile_pool(name="stp", bufs=6) as stp,
            tc.tile_pool(name="sp", bufs=8) as sp,
            tc.tile_pool(name="gtw", bufs=3) as gtw,
            tc.tile_pool(name="sm", bufs=3) as sm,
            tc.tile_pool(name="rows", bufs=4) as rows,
            tc.tile_pool(name="pbig", bufs=2, space="PSUM") as pbig,
            tc.tile_pool(name="pf", bufs=2, space="PSUM") as pf,
            tc.tile_pool(name="ps", bufs=3, space="PSUM") as psm,
            tc.tile_pool(name="ptr", bufs=1, space="PSUM") as ptr,
            tc.tile_pool(name="dram", bufs=1, space="DRAM") as dram,
        ):
            # ---- constants / persistent SBUF
            iotaT = const.tile([128, T * 128], F16)
            nc.sync.dma_start(out=iotaT[:], in_=iota_in[:, :])
            ident = const.tile([128, 128], F16)
            nc.sync.dma_start(out=ident[:], in_=ident_in[:, :])
            w1sb = const.tile([128, 2, EXTC], F16)
            nc.sync.dma_start(out=w1sb[:], in_=w1e.ap().rearrange("(k p) n -> p k n", p=128))
            w2sb = const.tile([128, 4, EXTC], F16)
            nc.sync.dma_start(out=w2sb[:], in_=w2e.ap().rearrange("(k p) n -> p k n", p=128))
            xTsb = const.tile([128, 2, NSLOT], F16)
            xTv = xT.ap().rearrange("(k p) n -> p k n", p=128)
            for c0 in range(0, NSLOT, NSLOT // 6):
                c1 = c0 + NSLOT // 6
                nc.sync.dma_start(out=xTsb[:, :, c0:c1], in_=xTv[:, :, c0:c1])
            idxsb = const.tile([128, NBLK, NIW], I16)
            nc.sync.dma_start(out=idxsb[:], in_=srcidx.ap().rearrange("b p s -> p b s"))
            dstsb2 = const.tile([128, NBLK, T, 2], F16)
            nc.sync.dma_start(
                out=dstsb2[:],
                in_=dst2.ap().rearrange("b p (t x) -> p b t x", x=2))
            if add_b1:
                b1sb = const.tile([128, D1], F32)
                nc.sync.dma_start(out=b1sb[:], in_=b1rep[:, :])
            if add_b2:
                b2sb = const.tile([128, LAT], F32)
                nc.sync.dma_start(out=b2sb[:], in_=b2rep[:, :])

            aldH1 = const.tile([128, NBLK, H], F16)
            aldH2 = const.tile([128, NBLK, H], F16)
            h1sb = const.tile([128, NBLK, D1], F16)

            bounce1 = dram.tile([NSLOT, ROW], F16)
            table1 = dram.tile([NT, ROW], F16, addr_space="Shared")
            bounce2 = dram.tile([NSLOT, ROW], F16)
            table2 = dram.tile([NT, ROW], F16, addr_space="Shared")

            Copy = mybir.ActivationFunctionType.Copy
            Relu = mybir.ActivationFunctionType.Relu
            Exp = mybir.ActivationFunctionType.Exp
            iseq = mybir.AluOpType.is_equal
            mult = mybir.AluOpType.mult

            def ag_all(bounce, table):
                nc.gpsimd.collective_compute(
                    "AllGather", mybir.AluOpType.bypass,
                    ins=[bounce.opt()], outs=[table.opt()],
                    replica_groups=[list(range(NCORES))])

            # ---------- phase A: xh1 shard -> bounce1 (pipelined)
            pstate = {}

            def PA0(j):
                # alternate PSUM pools: pf is idle during phase A, so this
                # doubles the pipeline depth
                pool, tag = (pbig, "pa") if j % 2 == 0 else (pf, "psF")
                pa = pool.tile([128, D1], F32, tag=tag)
                pbt = psm.tile([128, 128], F32, tag="small")
                pb = pbt[:, 0:8]
                for k in range(2):
                    lhs = xTsb[:, k, j * 128:(j + 1) * 128]
                    nc.tensor.matmul(pa[:], lhs, w1sb[:, k, 0:D1],
                                     start=(k == 0), stop=(k == 1))
                    nc.tensor.matmul(pb[:], lhs, w1sb[:, k, D1:EXTC],
                                     start=(k == 0), stop=(k == 1))
                pstate[j] = (pa, pb)

            def PA1(j):
                pa, pb = pstate.pop(j)
                row = rows.tile([128, EXTC - H], F16, tag="row")
                nc.scalar.activation(row[:, 0:D1], pa[:], Copy)
                nc.vector.tensor_copy(row[:, D1:D1 + H], pb[:, 0:H])
                nc.vector.tensor_copy(aldH1[:, j, :], pb[:, H:2 * H])
                nc.sync.dma_start(
                    out=bounce1[j * 128:(j + 1) * 128, 0:D1 + H], in_=row[:, :])

            PA0(0)
            for j in range(1, NBLK):
                PA0(j)
                PA1(j - 1)
            PA1(NBLK - 1)
            ag_all(bounce1, table1)

            # ---------- edge aggregation (software-pipelined) ----------
            def edge_phase(table, bounce, aldH, layer, after_block=None):
                state = {}

                def S0(j):
                    """prefetch: tile0 + gathers + St load + Sall build"""
                    G = gtp_g.tile([128, T, ROW], F16, tag="G")
                    nc.sync.dma_start(out=G[:, 0, 0:D1 + H],
                                      in_=bounce[j * 128:(j + 1) * 128, 0:D1 + H])
                    th = 1 + (T - 1 + 1) // 2
                    for qi, (t0, t1) in enumerate([(1, th), (th, T)]):
                        ni = (t1 - t0) * 128
                        nc.gpsimd.dma_gather(
                            G[:, t0:t1, :], table[:],
                            idxsb[:, j, (t0 - 1) * 8:(t1 - 1) * 8], ni, ni, ROW,
                            queue_num=(2 * j + qi) % 4)
                    # tile 0 of St is always the identity (self-loops); only
                    # tiles 1..T-1 are loaded. fp8: 0/1 are exact, halves DMA
                    Stal = stp.tile([128, T - 1, 128], mybir.dt.float8e4,
                                    tag="St")
                    nc.sync.dma_start(
                        out=Stal[:],
                        in_=stall_in[j].rearrange("p (t c) -> p t c", c=128))
                    Sall = sp.tile([128, T, 128], F16, tag="S")
                    nc.vector.tensor_tensor(
                        out=Sall[:],
                        in0=dstsb2[:, j].to_broadcast(
                            [128, T, 2, 64]).rearrange("p t x c -> p t c x"),
                        in1=iotaT[:].rearrange("p (t c) -> p t c", c=128),
                        op=iseq)
                    state[j] = dict(G=G, Stal=Stal, Sall=Sall)

                def S12(j):
                    """logits + exp + weighted features for block j"""
                    st = state[j]
                    G, Stal = st["G"], st["Stal"]
                    palz = psm.tile([128, 128], F32, tag="small")
                    pal = palz[:, 0:T * 4]
                    nc.tensor.matmul(pal[:], ident[:],
                                     G[:, :, D1:D1 + H],
                                     start=True, stop=False, skip_group_check=True)
                    for t in range(T):
                        lhs = ident[:] if t == 0 else Stal[:, t - 1, :]
                        nc.tensor.matmul(pal[:, 4 * t:4 * t + 4],
                                         lhs, aldH[:, j, :],
                                         start=False, stop=True,
                                         skip_group_check=True)
                    # leakyrelu: one scaled ACT copy off PSUM, then a DVE max
                    # against the PSUM logits (fast; avoids DVE tensor_scalar)
                    wln = sm.tile([128, T * 4], F32, tag="wln")
                    nc.scalar.activation(wln[:], pal[:], Copy, scale=NEG)
                    wl = sm.tile([128, T * 4], F32, tag="wl")
                    nc.vector.tensor_tensor(out=wl[:], in0=wln[:], in1=pal[:],
                                            op=mybir.AluOpType.max)
                    wexp2 = sm.tile([128, T * 4, 2], F16, tag="wexp2")
                    nc.scalar.activation(wexp2[:, :, 0], wl[:], Exp)
                    nc.scalar.activation(wexp2[:, :, 1], wl[:], Exp)
                    gt = gtw.tile([128, T, 4, 128], F16, tag="gt")
                    nc.vector.tensor_tensor(
                        out=gt[:],
                        in0=G[:, :, 0:D1].rearrange("p t (h c) -> p t h c", c=128),
                        in1=wexp2[:].to_broadcast(
                            [128, T * 4, 2, 64]).rearrange("p a x c -> p a c x"),
                        op=mult)
                    st.update(palz=palz, wexp2=wexp2, gt=gt)

                def S34(j):
                    """scatter + normalize + finalize (+ emit) for block j"""
                    st = state.pop(j)
                    Sall, palz, wexp2, gt = (st["Sall"], st["palz"],
                                             st["wexp2"], st["gt"])
                    psF = pf.tile([128, D1], F32, tag="psF")
                    for t in range(T):
                        nc.tensor.matmul(
                            psF[:], Sall[:, t, :],
                            gt[:, t].rearrange("p h c -> p (h c)"),
                            start=(t == 0), stop=(t == T - 1))
                        nc.tensor.matmul(
                            palz[:, 64:68], Sall[:, t, :],
                            wexp2[:, 4 * t:4 * t + 4, 0],
                            start=(t == 0), stop=(t == T - 1),
                            skip_group_check=True)
                    zz = sm.tile([128, 4], F32, tag="zz")
                    nc.scalar.activation(zz[:], palz[:, 64:68], Copy,
                                         scale=(1.0 if layer == 1 else 4.0))
                    rcp = sm.tile([128, 4], F32, tag="rcp")
                    nc.vector.reciprocal(rcp[:], zz[:])
                    if layer == 1:
                        if add_b1:
                            tmp = rows.tile([128, D1], F32, tag="tmpb")
                            for h in range(H):
                                nc.scalar.activation(
                                    tmp[:, h * 128:(h + 1) * 128],
                                    psF[:, h * 128:(h + 1) * 128], Copy,
                                    scale=rcp[:, h:h + 1])
                            nc.vector.tensor_add(tmp[:], tmp[:], b1sb[:])
                            nc.scalar.activation(h1sb[:, j, :], tmp[:], Relu)
                        else:
                            for h in range(H):
                                nc.scalar.activation(
                                    h1sb[:, j, h * 128:(h + 1) * 128],
                                    psF[:, h * 128:(h + 1) * 128], Relu,
                                    scale=rcp[:, h:h + 1])
                    else:
                        # rcp already = 1/(4z) via the zz scale above
                        htmp = rows.tile([128, 4, 128], F32, tag="htmp")
                        for h in range(H):
                            nc.scalar.activation(htmp[:, h, :],
                                                 psF[:, h * 128:(h + 1) * 128],
                                                 Copy, scale=rcp[:, h:h + 1])
                        o = rows.tile([128, LAT], F32, tag="o")
                        o2 = rows.tile([128, LAT], F32, tag="o2")
                        nc.vector.tensor_add(o[:], htmp[:, 0, :], htmp[:, 1, :])
                        nc.vector.tensor_add(o2[:], htmp[:, 2, :], htmp[:, 3, :])
                        nc.vector.tensor_add(o[:], o[:], o2[:])
                        if add_b2:
                            nc.vector.tensor_add(o[:], o[:], b2sb[:])
                        nc.sync.dma_start(out=out_ext[j * 128:(j + 1) * 128, :],
                                          in_=o[:])
                    if after_block is not None:
                        after_block(j)

                S0(0)
                S0(1)
                S0(2)
                for i in range(NBLK):
                    S12(i)
                    if i + 3 < NBLK:
                        S0(i + 3)
                    if i >= 1:
                        S34(i - 1)
                S34(NBLK - 1)

            def emit_C(j):
                h1T = rows.tile([128, 4, 128], F16, tag="h1T")
                pT = ptr.tile([128, D1], F16, tag="pt")
                for k in range(4):
                    nc.tensor.transpose(pT[:, k * 128:(k + 1) * 128],
                                        h1sb[:, j, k * 128:(k + 1) * 128], ident[:])
                nc.vector.tensor_copy(h1T[:], pT[:].rearrange("p (k c) -> p k c", c=128))
                pa = pbig.tile([128, D1], F32, tag="pa")
                pbt = psm.tile([128, 128], F32, tag="small")
                pb = pbt[:, 0:8]
                for k in range(4):
                    nc.tensor.matmul(pa[:], h1T[:, k, :], w2sb[:, k, 0:D1],
                                     start=(k == 0), stop=(k == 3))
                    nc.tensor.matmul(pb[:], h1T[:, k, :], w2sb[:, k, D1:EXTC],
                                     start=(k == 0), stop=(k == 3))
                row = rows.tile([128, EXTC - H], F16, tag="row")
                nc.scalar.activation(row[:, 0:D1], pa[:], Copy)
                nc.vector.tensor_copy(row[:, D1:D1 + H], pb[:, 0:H])
                nc.vector.tensor_copy(aldH2[:, j, :], pb[:, H:2 * H])
                nc.sync.dma_start(
                    out=bounce2[j * 128:(j + 1) * 128, 0:D1 + H], in_=row[:, :])

            edge_phase(table1, bounce1, aldH1, 1, after_block=emit_C)
            ag_all(bounce2, table2)
            edge_phase(table2, bounce2, aldH2, 2)

    nc.finalize()
    return nc


def _prep(inputs):
    x = np.asarray(inputs["x"], np.float32)
    ei = np.asarray(inputs["edge_index"], np.int64)
    W1 = np.asarray(inputs["W1"], np.float32)
    W2 = np.asarray(inputs["W2"], np.float32)
    src0, dst0 = ei[0], ei[1]

    # ---- balanced node -> (core, block, slot) assignment (LPT greedy)
    deg = np.bincount(dst0, minlength=N).astype(np.int64)
    NBINS = NCORES * NBLK
    order = np.argsort(-deg, kind="stable")
    counts = np.zeros(NBINS, np.int32)
    loads = np.zeros(NBINS, np.int64)
    binof = np.empty(N, np.int32)
    slotof = np.empty(N, np.int32)
    heap = [(0, 0, b) for b in range(NBINS)]
    heapq.heapify(heap)
    for v in order:
        while True:
            load, cnt, b = heapq.heappop(heap)
            if load == loads[b] and cnt == counts[b]:
                break
        binof[v] = b
        slotof[v] = counts[b]
        counts[b] += 1
        loads[b] += deg[v]
        if counts[b] < 128:
            heapq.heappush(heap, (int(loads[b]), int(counts[b]), b))
    T = 1 + int(np.ceil(loads.max() / 128.0))

    core_of = binof // NBLK
    blk_of = binof % NBLK
    lrow = blk_of * 128 + slotof                       # row within core
    trow = core_of.astype(np.int64) * NSLOT + lrow     # table row

    # ---- bucket input edges by destination bin
    eb = binof[dst0]
    es = trow[src0].astype(np.int64)
    edl = slotof[dst0].astype(np.int64)
    order_e = np.argsort(eb, kind="stable")
    eb_s, es_s, edl_s = eb[order_e], es[order_e], edl[order_e]
    starts = np.searchsorted(eb_s, np.arange(NBINS))
    ends = np.searchsorted(eb_s, np.arange(NBINS), side="right")

    NI = (T - 1) * 128
    srcidx_all = []
    dst2_all = []
    stall_all = []
    arange128 = np.arange(128)
    for c in range(NCORES):
        # Blocks 0..3 initialize the 4-deep G ring with real rows (pad idx 0
        # gathers row 0). Later blocks skip their pad tail (idx=-1): the
        # stale ring contents are finite real rows and dl=255 masks them.
        si = np.zeros((NBLK, NI), np.int16)
        dl = np.full((NBLK, T * 128), 255.0, np.float32)
        dl[:, 0:128] = arange128                      # tile0: self-loops
        for j in range(NBLK):
            b = c * NBLK + j
            s, e = starts[b], ends[b]
            cnt = e - s
            si[j, :cnt] = es_s[s:e].astype(np.int16)
            if j >= 4:
                si[j, cnt:] = -1
            dl[j, 128:128 + cnt] = edl_s[s:e]
        w = si.reshape(NBLK, NI // 16, 16).transpose(0, 2, 1)
        w = np.tile(w, (1, 8, 1))
        srcidx_all.append(np.ascontiguousarray(w))
        dlr = dl.reshape(NBLK, T, 128)
        # dst2[p, j, t*2:(t+1)*2] = dl[j, t*128+p] twice -> [NBLK,128,T*2]
        d2 = np.repeat(dlr.transpose(0, 2, 1), 2, axis=2).astype(np.float16)
        dst2_all.append(np.ascontiguousarray(d2))
        st = (dlr[:, None, 1:, :] == arange128[None, :, None, None])
        stall_all.append(np.ascontiguousarray(
            st.reshape(NBLK, 128, (T - 1) * 128).astype(
                mybir.dt.np(mybir.dt.float8e4))))

    w1ext = np.concatenate(
        [W1, _fold(W1, np.asarray(inputs["a_src1"], np.float32)),
         _fold(W1, np.asarray(inputs["a_dst1"], np.float32))], axis=1
    ).astype(np.float16)
    w2ext = np.concatenate(
        [W2, _fold(W2, np.asarray(inputs["a_src2"], np.float32)),
         _fold(W2, np.asarray(inputs["a_dst2"], np.float32))], axis=1
    ).astype(np.float16)

    iota = np.tile(np.arange(128, dtype=np.float16), (128, T))
    ident = np.eye(128, dtype=np.float16)

    b1 = np.asarray(inputs["b1"], np.float32)
    b2 = np.asarray(inputs["b2"], np.float32)
    add_b1 = bool(np.any(b1))
    add_b2 = bool(np.any(b2))

    in_maps = []
    for c in range(NCORES):
        m = (core_of == c)
        xs = np.zeros((IN, NSLOT), np.float16)
        xs[:, lrow[m]] = x[m].T.astype(np.float16)
        d = {
            "xT": xs,
            "w1e": w1ext,
            "w2e": w2ext,
            "srcidx": srcidx_all[c],
            "dst2": dst2_all[c],
            "stall_in": stall_all[c],
            "iota_in": iota,
            "ident_in": ident,
        }
        if add_b1:
            d["b1rep"] = np.tile(b1, (128, 1)).astype(np.float32)
        if add_b2:
            d["b2rep"] = np.tile(b2, (128, 1)).astype(np.float32)
        in_maps.append(d)
    return T, add_b1, add_b2, in_maps, core_of, lrow


def _run(inputs, trace=False):
    T, add_b1, add_b2, in_maps, core_of, lrow = _prep(inputs)
    key = (T, add_b1, add_b2)
    if key not in _CACHE:
        _CACHE[key] = _build(T, add_b1, add_b2)
    nc = _CACHE[key]
    res = bass_utils.run_bass_kernel_spmd(
        nc, in_maps, core_ids=list(range(NCORES)), trace=trace)
    big = np.concatenate([res.results[c]["out"] for c in range(NCORES)], axis=0)
    out = big[core_of.astype(np.int64) * NSLOT + lrow]
    return out.astype(np.float32), res


def kernel(**inputs):
    out, _ = _run(inputs, trace=False)
    return out
